# revision 22
# baseline (speedup 1.0000x reference)
"""GAT layer kernel for Trainium2, 8 NeuronCores.

Strategy (dst-sharded, zero collectives, identity-slot aggregation):
  - Host: append self-loops, split dst space into 8 equal ranges (one per
    core).  Per core, dst nodes are RELABELED in decreasing (in-degree+1)
    order; window w = labels [128w, 128w+128).  Edge (k-th incoming edge of
    label p, half hf) occupies gather slot (chunk, partition=p) where chunk
    enumerates k within the half-region.  Aggregation over a node's edges is
    then a plain sum over chunks at fixed partition -- NO onehot matmuls.
  - Phase 1 (replicated): htab[n] = [h(128) | a_src(4) | a_dst(4)] = x @
    W_ext via PE matmuls (bf16), rows strided 512B (dma_gather needs 256B
    multiples).  Tables split at S0=32512 so int16 gather indices reach every
    row; one sentinel row per table (h=0, a_src=-100) absorbs pad slots.
  - Phase 2 (per core, per window): one dma_gather per (window, half) on
    rotating SWDGE queues fetches rows into g[d-part, chunk, 512B].  Scores
    w = exp(leakyrelu(a_src + a_dst)) use the max-free softmax (bounded
    scores).  a_dst comes from the self-loop slot (chunk 0 of the node's own
    half).  num = sum_k w*h and den = sum_k w are DVE reduces; finalize is
    out = elu(layernorm(num/den + bias) * gamma + beta).  Output rows are in
    degree-sorted order; the host unpermutes.
"""

import numpy as np
import ml_dtypes

import concourse.bass as bass
import concourse.bacc as bacc
import concourse.mybir as mybir
import concourse.tile as tile
from concourse import library_config
from concourse.bass_utils import run_bass_kernel_spmd

BF16 = ml_dtypes.bfloat16
F32 = mybir.dt.float32
BF16_DT = mybir.dt.bfloat16
I16 = mybir.dt.int16

P = 128


class Cfg:
    def __init__(self, N=50000, E=1600000, DIN=256, DH=128, H=4, NCORES=8):
        self.N, self.E, self.DIN, self.DH, self.H = N, E, DIN, DH, H
        self.C = DH // H
        self.NCORES = NCORES
        self.ROW = DH + 2 * H               # 136 payload cols
        self.ROW_T = 256                    # stored row (512B stride)
        self.D_PER_CORE = N // NCORES       # 6250
        self.NWIN = (self.D_PER_CORE + P - 1) // P   # 49
        self.G1 = 12                        # node tiles per phase-1 group
        nt = (N + P - 1) // P
        self.NT = ((nt + self.G1 - 1) // self.G1) * self.G1   # 396
        self.NPAD = self.NT * P             # 50688
        self.KD = (DIN + P - 1) // P        # matmul k-chunks (2)
        self.WOUT = 4                       # windows batched per output store
        self.S0 = 32512                     # htab0 rows (254*128, idx<=32512)
        self.GSUB = 8                       # max chunks per gather op

    NEG = 0.2
    LN_EPS = 1e-5
    DEN_EPS = 1e-20
    SENT_A = -100.0


DEFAULT_CFG = Cfg()


def _wrap16(flat):
    """int16 index list -> dma_gather idxs layout [128, n/16]."""
    n = len(flat)
    a = flat.reshape(n // 16, 16).T          # [16, n/16]
    return np.tile(a, (8, 1))                # replicated to 128 partitions


# --------------------------------------------------------------------------
# Host-side preparation (layout only; all FLOPs on x stay on device)
# --------------------------------------------------------------------------

def host_prep(cfg, x, edge_index, W, att_src, att_dst, bias, ln_gamma, ln_beta):
    N, DIN, DH, H, C = cfg.N, cfg.DIN, cfg.DH, cfg.H, cfg.C
    NC, DPC, NWIN, S0 = cfg.NCORES, cfg.D_PER_CORE, cfg.NWIN, cfg.S0
    N1 = cfg.NPAD - S0                       # htab1 node rows

    x = np.asarray(x, np.float32)
    W = np.asarray(W, np.float32)
    att_src = np.asarray(att_src, np.float32)
    att_dst = np.asarray(att_dst, np.float32)

    Msrc = np.zeros((DH, H), np.float32)
    Mdst = np.zeros((DH, H), np.float32)
    for h in range(H):
        Msrc[h * C:(h + 1) * C, h] = att_src[h]
        Mdst[h * C:(h + 1) * C, h] = att_dst[h]
    W_ext = np.concatenate([W, W @ Msrc, W @ Mdst], axis=1)  # [DIN, ROW]
    W16 = np.ascontiguousarray(W_ext).astype(BF16)

    xT = np.zeros((DIN, cfg.NPAD), np.float32)
    xT[:, :N] = x.T
    xT16 = xT.astype(BF16)

    src = np.asarray(edge_index[0], np.int64)
    dst = np.asarray(edge_index[1], np.int64)

    gbb = np.stack([np.asarray(ln_gamma, np.float32),
                    np.asarray(ln_beta, np.float32),
                    np.asarray(bias, np.float32)], 0)
    # trivial affine params (the common init) let the kernel skip 3 DVE ops
    triv = bool(np.all(gbb[0] == 1.0) and np.all(gbb[1] == 0.0)
                and np.all(gbb[2] == 0.0))
    # sentinel rows: h = 0, a_src = SENT_A, a_dst = 0
    sent = np.zeros((2, cfg.ROW), np.float32)
    sent[:, DH:DH + H] = cfg.SENT_A
    sent16 = sent.astype(BF16)

    # ---- per-core edge layout ----
    core_of = dst // DPC
    per_core = []   # (order, c0, c1) per core
    for c in range(NC):
        m = core_of == c
        s_c = src[m]
        d_loc = dst[m] - c * DPC
        # self-loops for this core's dst range
        own = np.arange(c * DPC, (c + 1) * DPC, dtype=np.int64)
        s_all = np.concatenate([own, s_c])
        d_all = np.concatenate([own - c * DPC, d_loc])
        is_self = np.zeros(len(s_all), np.int8)
        is_self[:DPC] = 1      # used to force self-loop to slot k=0
        hf = (s_all >= S0).astype(np.int64)
        c0 = np.bincount(d_all[hf == 0], minlength=DPC)
        c1 = np.bincount(d_all[hf == 1], minlength=DPC)
        per_core.append([None, c0, c1, s_all, d_all, hf, is_self])

    # window packing: band by c0 (width B), sort by c1 within band -- windows
    # get a tight max-c0 and max-c1.  Scan B for the best global chunk count.
    def pack(B):
        K0s = np.zeros(NWIN, np.int64)
        K1s = np.zeros(NWIN, np.int64)
        orders = []
        for c in range(NC):
            c0, c1 = per_core[c][1], per_core[c][2]
            order = np.lexsort((-c1, -(c0 // B)))
            orders.append(order)
            c0s = np.zeros(NWIN * P, np.int64)
            c1s = np.zeros(NWIN * P, np.int64)
            c0s[:DPC] = c0[order]
            c1s[:DPC] = c1[order]
            K0s = np.maximum(K0s, c0s.reshape(NWIN, P).max(1))
            K1s = np.maximum(K1s, c1s.reshape(NWIN, P).max(1))
        return (K0s + K1s).sum(), orders, K0s, K1s

    best = min((pack(B) for B in (3, 4, 5, 6, 8, 10)), key=lambda t: t[0])
    _, orders, K0s, K1s = best
    for c in range(NC):
        per_core[c][0] = orders[c]
    KWs = K0s + K1s
    offs = np.zeros(NWIN + 1, np.int64)
    np.cumsum(KWs, out=offs[1:])
    TOTCH = int(offs[-1])

    in_maps = []
    out_perms = []
    for c in range(NC):
        order, c0, c1, s_all, d_all, hf, is_self = per_core[c]
        label_of = np.empty(DPC, np.int64)
        label_of[order] = np.arange(DPC)
        lab = label_of[d_all]                      # label per edge
        w_of = lab >> 7
        p_of = lab & 127
        # position of each edge within its (label, half) group, self first
        key = ((lab * 2 + hf) << 1) | (1 - is_self).astype(np.int64)
        eo = np.argsort(key, kind="stable")
        ks = key[eo] >> 1                          # group id = lab*2+hf
        starts = np.searchsorted(ks, np.arange(DPC * 2) * 1)
        # position within group
        grp_start = starts[ks]
        pos = np.arange(len(eo)) - grp_start
        # chunk index within window
        k0w = K0s[w_of[eo]]
        chunk = np.where(hf[eo] == 0, pos, k0w + pos)
        gchunk = offs[w_of[eo]] + chunk
        slot = gchunk * P + p_of[eo]
        rowid = np.where(hf[eo] == 0, s_all[eo], s_all[eo] - S0)

        flat = np.empty(TOTCH * P, np.int16)
        # defaults: sentinel of the chunk's half
        half1_chunk = np.zeros(TOTCH, bool)
        for w in range(NWIN):
            half1_chunk[offs[w] + K0s[w]:offs[w + 1]] = True
        flat.reshape(TOTCH, P)[~half1_chunk] = S0        # sentinel htab0
        flat.reshape(TOTCH, P)[half1_chunk] = N1         # sentinel htab1
        flat[slot] = rowid.astype(np.int16)

        si16 = np.zeros((P, TOTCH * 8), np.int16)
        for w in range(NWIN):
            a, b = int(offs[w]), int(offs[w] + K0s[w])
            if b > a:
                si16[:, a * 8:b * 8] = _wrap16(flat[a * P:b * P])
            a, b = int(offs[w] + K0s[w]), int(offs[w + 1])
            if b > a:
                si16[:, a * 8:b * 8] = _wrap16(flat[a * P:b * P])

        # window-node half masks (m0=1.0 if node id < S0 -> self-loop slot in
        # half0; m1 = 1 - m0).  Pad labels (>= DPC) point at sentinel rows
        # whose a_dst is 0 -- either half works.
        node_of_label = np.zeros(NWIN * P, np.int64)
        node_of_label[:DPC] = order + c * DPC
        glob = node_of_label.reshape(NWIN, P).T       # [P, NWIN]
        m0h = (glob < S0).astype(np.float32)
        m0 = np.stack([m0h, 1.0 - m0h], axis=2)       # [P, NWIN, 2]

        in_maps.append({
            "xT": xT16,
            "wext": W16,
            "si16": si16,
            "m0": np.ascontiguousarray(m0.reshape(P, 2 * NWIN)),
            "gbb": gbb,
            "sent": sent16,
        })
        out_perms.append(order + c * DPC)
    return in_maps, tuple(int(v) for v in K0s), tuple(int(v) for v in K1s), \
        TOTCH, out_perms, triv


# --------------------------------------------------------------------------
# Bass kernel builder (identical NEFF for all cores)
# --------------------------------------------------------------------------

def build_nc(cfg, K0s, K1s, TOTCH, triv=False, dbg=None):
    N, DIN, DH, H, C = cfg.N, cfg.DIN, cfg.DH, cfg.H, cfg.C
    ROW, ROW_T, NWIN, NT, NPAD, G1, KD = (cfg.ROW, cfg.ROW_T, cfg.NWIN,
                                          cfg.NT, cfg.NPAD, cfg.G1, cfg.KD)
    S0 = cfg.S0
    N1 = NPAD - S0
    KWs = [a + b for a, b in zip(K0s, K1s)]
    KWMAX = max(KWs)
    offs = [0]
    for v in KWs:
        offs.append(offs[-1] + v)
    NG1 = NT // G1
    BPG = (G1 + 2) // 3
    WOUT = cfg.WOUT

    nc = bacc.Bacc("TRN2", num_swdge_queues=4, dynamic_dma_scratch_size=32768)
    xT_d = nc.dram_tensor("xT", [DIN, NPAD], BF16_DT, kind="ExternalInput")
    w_d = nc.dram_tensor("wext", [DIN, ROW], BF16_DT, kind="ExternalInput")
    si_d = nc.dram_tensor("si16", [P, TOTCH * 8], I16, kind="ExternalInput")
    m0_d = nc.dram_tensor("m0", [P, 2 * NWIN], F32, kind="ExternalInput")
    gbb_d = nc.dram_tensor("gbb", [3, DH], F32, kind="ExternalInput")
    sent_d = nc.dram_tensor("sent", [2, ROW], BF16_DT, kind="ExternalInput")
    y_d = nc.dram_tensor("y", [NWIN * P, DH], F32, kind="ExternalOutput")
    htab0 = nc.dram_tensor("htab0", [S0 + P, ROW_T], BF16_DT, kind="Internal")
    htab1 = nc.dram_tensor("htab1", [N1 + P, ROW_T], BF16_DT, kind="Internal")

    qrr = [0]

    def next_q():
        q = qrr[0]
        qrr[0] = (q + 1) % 4
        return q

    nc.gpsimd.load_library(library_config.mlp)
    with tile.TileContext(nc) as tc:
        with tc.tile_pool(name="const", bufs=1) as const, \
             tc.tile_pool(name="mp", bufs=2) as mp:
            wt = const.tile([P, KD, ROW], BF16_DT)
            for k in range(KD):
                nc.sync.dma_start(out=wt[:, k, :], in_=w_d[k * P:(k + 1) * P, :])
            si_t = const.tile([P, TOTCH * 8], I16)
            nc.sync.dma_start(out=si_t[:], in_=si_d[:])
            m0_t = const.tile([P, 2 * NWIN], F32)
            nc.sync.dma_start(out=m0_t[:], in_=m0_d[:])
            gam_t = const.tile([P, DH], F32)
            bet_t = const.tile([P, DH], F32)
            bia_t = const.tile([P, DH], F32)
            for t, i in ((gam_t, 0), (bet_t, 1), (bia_t, 2)):
                a = gbb_d[i, :]
                src_ap = bass.AP(a.tensor, a.offset, [[0, P], [1, DH]])
                nc.gpsimd.dma_start(out=t[:], in_=src_ap)
            eps_t = const.tile([P, 1], F32)
            nc.vector.memset(eps_t[:], cfg.LN_EPS)
            neg_t = const.tile([P, 1], F32)
            nc.vector.memset(neg_t[:], cfg.NEG)
            sent_t = const.tile([2, ROW], BF16_DT)
            nc.sync.dma_start(out=sent_t[:], in_=sent_d[:])

            # ---- phase 1: htab[:, 0:136] = x @ W_ext ----
            nc.gpsimd.dma_start(out=htab0[S0:S0 + 1, 0:ROW], in_=sent_t[0:1, :])
            nc.gpsimd.dma_start(out=htab1[N1:N1 + 1, 0:ROW], in_=sent_t[1:2, :])
            with (
                tc.tile_pool(name="xp", bufs=2) as xp,
                tc.tile_pool(name="stg", bufs=2) as stg,
                tc.tile_pool(name="ps1", bufs=2, space="PSUM") as ps1,
            ):
                for g in range(NG1):
                    xk = xp.tile([P, KD, G1 * P], BF16_DT)
                    for k in range(KD):
                        nc.sync.dma_start(
                            out=xk[:, k, :],
                            in_=xT_d[k * P:(k + 1) * P,
                                     g * G1 * P:(g + 1) * G1 * P])
                    ps = ps1.tile([P, BPG, 512], F32, tag="ps1")
                    for i in range(G1):
                        pslice = ps[:, i // 3, (i % 3) * ROW:(i % 3 + 1) * ROW]
                        for k in range(KD):
                            nc.tensor.matmul(
                                pslice, lhsT=xk[:, k, i * P:(i + 1) * P],
                                rhs=wt[:, k, :],
                                start=(k == 0), stop=(k == KD - 1))
                    stage = stg.tile([P, G1, ROW], BF16_DT, tag="stage")
                    nc.scalar.copy(
                        out=stage[:].rearrange("p (b t) r -> p b t r", t=3),
                        in_=ps[:, :, 0:3 * ROW].rearrange(
                            "p b (t r) -> p b t r", r=ROW))
                    n0 = g * G1 * P
                    n1 = n0 + G1 * P
                    if n1 <= S0 or n0 >= S0:
                        t, o = (htab0, n0) if n1 <= S0 else (htab1, n0 - S0)
                        dst_ap = t[o:o + G1 * P, 0:ROW].rearrange(
                            "(b p) r -> p b r", p=P)
                        nc.gpsimd.dma_start(out=dst_ap, in_=stage[:])
                    else:
                        bs = (S0 - n0) // P   # boundary is 128-aligned
                        dst_ap = htab0[n0:S0, 0:ROW].rearrange(
                            "(b p) r -> p b r", p=P)
                        nc.gpsimd.dma_start(out=dst_ap, in_=stage[:, 0:bs, :])
                        dst_ap = htab1[0:n1 - S0, 0:ROW].rearrange(
                            "(b p) r -> p b r", p=P)
                        nc.gpsimd.dma_start(out=dst_ap, in_=stage[:, bs:, :])

            if dbg == "phase1":
                with tc.tile_pool(name="dbgp", bufs=2) as dbgp:
                    for w in range(NWIN):
                        t = dbgp.tile([P, DH], F32, tag="dbg")
                        nc.gpsimd.dma_start(out=t[:],
                                            in_=htab0[w * P:(w + 1) * P, 0:DH])
                        nc.gpsimd.dma_start(out=y_d[w * P:(w + 1) * P, :],
                                            in_=t[:])
            if dbg is None:
              # ---- phase 2 ----
              with (
                  tc.tile_pool(name="rp", bufs=2) as rp,
                  tc.tile_pool(name="wp", bufs=2) as wp,
                  tc.tile_pool(name="fp", bufs=2) as fp,
                  tc.tile_pool(name="outp", bufs=2) as outp,
              ):
                  ost = None
                  for w in range(NWIN):
                      K0, K1 = K0s[w], K1s[w]
                      KW = KWs[w]
                      off = offs[w]
                      g_main = mp.tile([P, KWMAX, ROW_T], BF16_DT, tag="gm")
                      # gathers: one span per half, split at GSUB chunks
                      for base, kn, htb, span in ((0, K0, htab0, S0 + P),
                                                  (K0, K1, htab1, N1 + P)):
                          k = 0
                          while k < kn:
                              ke = min(k + cfg.GSUB, kn)
                              a = base + k
                              b = base + ke
                              nc.gpsimd.dma_gather(
                                  out_ap=g_main[:, a:b, :],
                                  in_ap=htb[0:span, :],
                                  idxs_ap=si_t[:, (off + a) * 8:(off + b) * 8],
                                  num_idxs=(b - a) * P,
                                  num_idxs_reg=(b - a) * P,
                                  elem_size=ROW_T,
                                  queue_num=next_q())
                              k = ke

                      # a_dst for the window's nodes: self-loop slot is chunk 0
                      # of the node's own half (blend by m0 mask)
                      adw = wp.tile([P, H], F32, tag="adw")
                      if K0 and K1:
                          nc.vector.tensor_scalar(
                              out=adw[:], in0=g_main[:, 0, DH + H:DH + 2 * H],
                              scalar1=m0_t[:, 2 * w:2 * w + 1], scalar2=None,
                              op0=mybir.AluOpType.mult)
                          nc.vector.scalar_tensor_tensor(
                              out=adw[:],
                              in0=g_main[:, K0, DH + H:DH + 2 * H],
                              scalar=m0_t[:, 2 * w + 1:2 * w + 2],
                              in1=adw[:], op0=mybir.AluOpType.mult,
                              op1=mybir.AluOpType.add)
                      else:
                          nc.vector.tensor_copy(
                              out=adw[:],
                              in_=g_main[:, 0, DH + H:DH + 2 * H])

                      # scores: sc = a_src + a_dst (bcast over chunks), then
                      # w = exp(leakyrelu(sc)) on the scalar engine
                      sc = wp.tile([P, KWMAX, H], F32, tag="sc")
                      a = adw[:]
                      ad_b = bass.AP(a.tensor, a.offset,
                                     [a.ap[0], [0, KW], a.ap[1]])
                      nc.vector.tensor_tensor(
                          out=sc[:, :KW, :], in0=g_main[:, :KW, DH:DH + H],
                          in1=ad_b, op=mybir.AluOpType.add)
                      sc2 = wp.tile([P, KWMAX, H], F32, tag="sc2")
                      nc.scalar.activation(out=sc2[:, :KW, :],
                                           in_=sc[:, :KW, :],
                                           func=mybir.ActivationFunctionType.Prelu,
                                           alpha=neg_t[:])
                      wf = wp.tile([P, KWMAX, H], BF16_DT, tag="wf")
                      nc.scalar.activation(out=wf[:, :KW, :], in_=sc2[:, :KW, :],
                                           func=mybir.ActivationFunctionType.Exp)

                      # wrep = w broadcast to feature width (scalar engine, so
                      # the big DVE multiply below gets contiguous operands)
                      wrep = rp.tile([P, KWMAX, DH], BF16_DT, tag="wrep")
                      a2 = wf[:, :KW, :]
                      w_b = bass.AP(a2.tensor, a2.offset,
                                    [a2.ap[0], a2.ap[1], a2.ap[2], [0, C]])
                      nc.scalar.copy(
                          out=wrep[:, :KW, :].rearrange("p k (h c) -> p k h c",
                                                        h=H),
                          in_=w_b)
                      # rhs = h * w
                      rhs = rp.tile([P, KWMAX, DH], BF16_DT, tag="rhs")
                      nc.vector.tensor_tensor(
                          out=rhs[:, :KW, :], in0=g_main[:, :KW, 0:DH],
                          in1=wrep[:, :KW, :], op=mybir.AluOpType.mult)

                      # num = sum_k rhs: halving tree, contiguous operands
                      num = fp.tile([P, DH], F32, tag="num")
                      n = KW
                      while n > 2:
                          hh = n // 2          # fold tail onto head
                          ce = n - hh          # ceil
                          nc.vector.tensor_tensor(
                              out=rhs[:, :hh, :], in0=rhs[:, :hh, :],
                              in1=rhs[:, ce:ce + hh, :],
                              op=mybir.AluOpType.add)
                          n = ce
                      if n == 2:
                          nc.vector.tensor_tensor(
                              out=num[:], in0=rhs[:, 0, :], in1=rhs[:, 1, :],
                              op=mybir.AluOpType.add)
                      else:
                          nc.vector.tensor_copy(out=num[:], in_=rhs[:, 0, :])
                      # den = sum_k w (small strided reduce)
                      den = fp.tile([P, H], F32, tag="den")
                      nc.vector.tensor_reduce(
                          out=den[:],
                          in_=wf[:, :KW, :].rearrange("p k h -> p h k"),
                          axis=mybir.AxisListType.X, op=mybir.AluOpType.add)

                      nc.vector.tensor_scalar_add(out=den[:], in0=den[:],
                                                  scalar1=cfg.DEN_EPS)
                      nc.vector.reciprocal(out=den[:], in_=den[:])
                      y = fp.tile([P, DH], F32, tag="y")
                      da = den[:]
                      den_b = bass.AP(da.tensor, da.offset,
                                      [da.ap[0], da.ap[1], [0, C]])
                      nc.vector.tensor_tensor(
                          out=y[:].rearrange("p (h c) -> p h c", h=H),
                          in0=num[:].rearrange("p (h c) -> p h c", h=H),
                          in1=den_b, op=mybir.AluOpType.mult)
                      if not triv:
                          nc.vector.tensor_tensor(out=y[:], in0=y[:],
                                                  in1=bia_t[:],
                                                  op=mybir.AluOpType.add)
                      st = fp.tile([P, 6], F32, tag="st")
                      nc.vector.bn_stats(out=st[:], in_=y[:])
                      mv = fp.tile([P, 2], F32, tag="mv")
                      nc.vector.bn_aggr(out=mv[:], in_=st[:])
                      nc.scalar.activation(out=mv[:, 1:2], in_=mv[:, 1:2],
                                           func=mybir.ActivationFunctionType.Sqrt,
                                           bias=eps_t[:])
                      nc.vector.reciprocal(out=mv[:, 1:2], in_=mv[:, 1:2])
                      # z = (y - mu) * rstd on the scalar engine:
                      # scale = rstd, bias = -mu * rstd
                      nmr = fp.tile([P, 1], F32, tag="nmr")
                      nc.vector.scalar_tensor_tensor(
                          out=nmr[:], in0=mv[:, 0:1], scalar=-1.0,
                          in1=mv[:, 1:2], op0=mybir.AluOpType.mult,
                          op1=mybir.AluOpType.mult)
                      z = fp.tile([P, DH], F32, tag="z")
                      nc.scalar.activation(out=z[:], in_=y[:],
                                           func=mybir.ActivationFunctionType.Identity,
                                           scale=mv[:, 1:2], bias=nmr[:])
                      if not triv:
                          nc.vector.tensor_tensor(out=z[:], in0=z[:],
                                                  in1=gam_t[:],
                                                  op=mybir.AluOpType.mult)
                          nc.vector.tensor_tensor(out=z[:], in0=z[:],
                                                  in1=bet_t[:],
                                                  op=mybir.AluOpType.add)
                      # elu: out = (relu(z) - 1) + exp(min(z, 0))
                      zm = fp.tile([P, DH], F32, tag="zm")
                      nc.scalar.activation(out=zm[:], in_=z[:],
                                           func=mybir.ActivationFunctionType.Relu)
                      zn = fp.tile([P, DH], F32, tag="zn")
                      nc.vector.tensor_scalar(out=zn[:], in0=z[:], scalar1=0.0,
                                              scalar2=None,
                                              op0=mybir.AluOpType.min)
                      te = fp.tile([P, DH], F32, tag="te")
                      nc.scalar.activation(out=te[:], in_=zn[:],
                                           func=mybir.ActivationFunctionType.Exp)
                      if w % WOUT == 0:
                          ost = outp.tile([P, WOUT, DH], F32, tag="ost")
                      nc.vector.scalar_tensor_tensor(
                          out=ost[:, w % WOUT, :], in0=zm[:], scalar=-1.0,
                          in1=te[:], op0=mybir.AluOpType.add,
                          op1=mybir.AluOpType.add)
                      if w % WOUT == WOUT - 1 or w == NWIN - 1:
                          w0 = (w // WOUT) * WOUT
                          nb = w - w0 + 1
                          dst_ap = y_d[w0 * P:(w + 1) * P, :].rearrange(
                              "(b p) r -> p b r", p=P)
                          nc.scalar.dma_start(out=dst_ap, in_=ost[:, :nb, :])

    nc.compile()
    return nc


# --------------------------------------------------------------------------
# Entry point
# --------------------------------------------------------------------------

_CACHE = {}


def kernel(x, edge_index, W, att_src, att_dst, bias, ln_gamma, ln_beta,
           cfg=DEFAULT_CFG, trace=False, dbg=None):
    in_maps, K0s, K1s, TOTCH, out_perms, triv = host_prep(
        cfg, x, edge_index, W, att_src, att_dst, bias, ln_gamma, ln_beta)
    key = (cfg.N, cfg.E, K0s, K1s, TOTCH, triv, dbg)
    if key not in _CACHE:
        _CACHE[key] = build_nc(cfg, K0s, K1s, TOTCH, triv=triv, dbg=dbg)
    nc = _CACHE[key]
    r = run_bass_kernel_spmd(nc, in_maps, core_ids=list(range(cfg.NCORES)),
                             trace=trace)
    out = np.empty((cfg.N, cfg.DH), np.float32)
    for c in range(cfg.NCORES):
        out[out_perms[c]] = r.results[c]["y"][:cfg.D_PER_CORE]
    kernel.last_result = r
    return out


# revision 30
# speedup vs baseline: 1.1298x; 1.1298x over previous
"""GAT layer kernel for Trainium2, 8 NeuronCores.

Strategy (dst-sharded, zero collectives, identity-slot aggregation):
  - Host: append self-loops, split dst space into 8 equal ranges (one per
    core).  Per core, dst nodes are RELABELED in decreasing (in-degree+1)
    order; window w = labels [128w, 128w+128).  Edge (k-th incoming edge of
    label p, half hf) occupies gather slot (chunk, partition=p) where chunk
    enumerates k within the half-region.  Aggregation over a node's edges is
    then a plain sum over chunks at fixed partition -- NO onehot matmuls.
  - Phase 1 (replicated): htab[n] = [h(128) | a_src(4) | a_dst(4)] = x @
    W_ext via PE matmuls (bf16), rows strided 512B (dma_gather needs 256B
    multiples).  Tables split at S0=32512 so int16 gather indices reach every
    row; one sentinel row per table (h=0, a_src=-100) absorbs pad slots.
  - Phase 2 (per core, per window): one dma_gather per (window, half) on
    rotating SWDGE queues fetches rows into g[d-part, chunk, 512B].  Scores
    w = exp(leakyrelu(a_src + a_dst)) use the max-free softmax (bounded
    scores).  a_dst comes from the self-loop slot (chunk 0 of the node's own
    half).  num = sum_k w*h and den = sum_k w are DVE reduces; finalize is
    out = elu(layernorm(num/den + bias) * gamma + beta).  Output rows are in
    degree-sorted order; the host unpermutes.
"""

import numpy as np
import ml_dtypes

import concourse.bass as bass
import concourse.bacc as bacc
import concourse.mybir as mybir
import concourse.tile as tile
from concourse import library_config
from concourse.bass_utils import run_bass_kernel_spmd

BF16 = ml_dtypes.bfloat16
F32 = mybir.dt.float32
BF16_DT = mybir.dt.bfloat16
I16 = mybir.dt.int16

P = 128


class Cfg:
    def __init__(self, N=50000, E=1600000, DIN=256, DH=128, H=4, NCORES=8):
        self.N, self.E, self.DIN, self.DH, self.H = N, E, DIN, DH, H
        self.C = DH // H
        self.NCORES = NCORES
        self.ROW = DH + 2 * H               # 136 payload cols
        self.ROW_T = 256                    # stored row (512B stride)
        self.D_PER_CORE = N // NCORES       # 6250
        self.NWIN = (self.D_PER_CORE + P - 1) // P   # 49
        self.G1 = 12                        # node tiles per phase-1 group
        nt = (N + P - 1) // P
        self.NT = ((nt + self.G1 - 1) // self.G1) * self.G1   # 396
        self.NPAD = self.NT * P             # 50688
        self.KD = (DIN + P - 1) // P        # matmul k-chunks (2)
        self.WOUT = 4                       # windows batched per output store
        self.S0 = 32512                     # htab0 rows (254*128, idx<=32512)
        self.GSUB = 8                       # max chunks per gather op

    NEG = 0.2
    LN_EPS = 1e-5
    DEN_EPS = 1e-20
    SENT_A = -100.0


DEFAULT_CFG = Cfg()
TRIM = True


def _wrap16(flat):
    """int16 index list -> dma_gather idxs layout [128, n/16]."""
    n = len(flat)
    a = flat.reshape(n // 16, 16).T          # [16, n/16]
    return np.tile(a, (8, 1))                # replicated to 128 partitions


# --------------------------------------------------------------------------
# Host-side preparation (layout only; all FLOPs on x stay on device)
# --------------------------------------------------------------------------

def host_prep(cfg, x, edge_index, W, att_src, att_dst, bias, ln_gamma, ln_beta):
    N, DIN, DH, H, C = cfg.N, cfg.DIN, cfg.DH, cfg.H, cfg.C
    NC, DPC, NWIN, S0 = cfg.NCORES, cfg.D_PER_CORE, cfg.NWIN, cfg.S0
    N1 = cfg.NPAD - S0                       # htab1 node rows

    x = np.asarray(x, np.float32)
    W = np.asarray(W, np.float32)
    att_src = np.asarray(att_src, np.float32)
    att_dst = np.asarray(att_dst, np.float32)

    Msrc = np.zeros((DH, H), np.float32)
    Mdst = np.zeros((DH, H), np.float32)
    for h in range(H):
        Msrc[h * C:(h + 1) * C, h] = att_src[h]
        Mdst[h * C:(h + 1) * C, h] = att_dst[h]
    W_ext = np.concatenate([W, W @ Msrc, W @ Mdst], axis=1)  # [DIN, ROW]
    W16 = np.ascontiguousarray(W_ext).astype(BF16)

    xT = np.zeros((DIN, cfg.NPAD), np.float32)
    xT[:, :N] = x.T
    xT16 = xT.astype(BF16)

    src = np.asarray(edge_index[0], np.int64)
    dst = np.asarray(edge_index[1], np.int64)

    gbb = np.stack([np.asarray(ln_gamma, np.float32),
                    np.asarray(ln_beta, np.float32),
                    np.asarray(bias, np.float32)], 0)
    # trivial affine params (the common init) let the kernel skip 3 DVE ops
    triv = bool(np.all(gbb[0] == 1.0) and np.all(gbb[1] == 0.0)
                and np.all(gbb[2] == 0.0))
    # sentinel rows: h = 0, a_src = SENT_A, a_dst = 0
    sent = np.zeros((2, cfg.ROW), np.float32)
    sent[:, DH:DH + H] = cfg.SENT_A
    sent16 = sent.astype(BF16)

    # ---- per-core edge layout ----
    core_of = dst // DPC
    per_core = []   # (order, c0, c1) per core
    for c in range(NC):
        m = core_of == c
        s_c = src[m]
        d_loc = dst[m] - c * DPC
        # self-loops for this core's dst range
        own = np.arange(c * DPC, (c + 1) * DPC, dtype=np.int64)
        s_all = np.concatenate([own, s_c])
        d_all = np.concatenate([own - c * DPC, d_loc])
        is_self = np.zeros(len(s_all), np.int8)
        is_self[:DPC] = 1      # used to force self-loop to slot k=0
        hf = (s_all >= S0).astype(np.int64)
        c0 = np.bincount(d_all[hf == 0], minlength=DPC)
        c1 = np.bincount(d_all[hf == 1], minlength=DPC)
        per_core.append([None, c0, c1, s_all, d_all, hf, is_self])

    # window packing: band by c0 (width B), sort by c1 within band -- windows
    # get a tight max-c0 and max-c1.  Scan B for the best global chunk count.
    def pack(B):
        K0s = np.zeros(NWIN, np.int64)
        K1s = np.zeros(NWIN, np.int64)
        orders = []
        for c in range(NC):
            c0, c1 = per_core[c][1], per_core[c][2]
            order = np.lexsort((-c1, -(c0 // B)))
            orders.append(order)
            c0s = np.zeros(NWIN * P, np.int64)
            c1s = np.zeros(NWIN * P, np.int64)
            c0s[:DPC] = c0[order]
            c1s[:DPC] = c1[order]
            K0s = np.maximum(K0s, c0s.reshape(NWIN, P).max(1))
            K1s = np.maximum(K1s, c1s.reshape(NWIN, P).max(1))
        return (K0s + K1s).sum(), orders, K0s, K1s

    best = min((pack(B) for B in (3, 4, 5, 6, 8, 10)), key=lambda t: t[0])
    _, orders, K0s, K1s = best
    for c in range(NC):
        per_core[c][0] = orders[c]
    KWs = K0s + K1s
    offs = np.zeros(NWIN + 1, np.int64)
    np.cumsum(KWs, out=offs[1:])
    TOTCH = int(offs[-1])

    in_maps = []
    out_perms = []
    for c in range(NC):
        order, c0, c1, s_all, d_all, hf, is_self = per_core[c]
        label_of = np.empty(DPC, np.int64)
        label_of[order] = np.arange(DPC)
        lab = label_of[d_all]                      # label per edge
        w_of = lab >> 7
        p_of = lab & 127
        # position of each edge within its (label, half) group, self first
        key = ((lab * 2 + hf) << 1) | (1 - is_self).astype(np.int64)
        eo = np.argsort(key, kind="stable")
        ks = key[eo] >> 1                          # group id = lab*2+hf
        starts = np.searchsorted(ks, np.arange(DPC * 2) * 1)
        # position within group
        grp_start = starts[ks]
        pos = np.arange(len(eo)) - grp_start
        # chunk index within window
        k0w = K0s[w_of[eo]]
        chunk = np.where(hf[eo] == 0, pos, k0w + pos)
        gchunk = offs[w_of[eo]] + chunk
        slot = gchunk * P + p_of[eo]
        rowid = np.where(hf[eo] == 0, s_all[eo], s_all[eo] - S0)

        flat = np.empty(TOTCH * P, np.int16)
        # defaults: sentinel of the chunk's half; chunks beyond this core's
        # own per-window max count are all-pad -> negative idx (the SWDGE
        # ucode suffix-trims them, skipping both descriptors and fetch)
        half1_chunk = np.zeros(TOTCH, bool)
        for w in range(NWIN):
            half1_chunk[offs[w] + K0s[w]:offs[w + 1]] = True
        flat.reshape(TOTCH, P)[~half1_chunk] = S0        # sentinel htab0
        flat.reshape(TOTCH, P)[half1_chunk] = N1         # sentinel htab1
        c0s = np.zeros(NWIN * P, np.int64)
        c1s = np.zeros(NWIN * P, np.int64)
        c0s[:DPC] = c0[order]
        c1s[:DPC] = c1[order]
        ck0 = c0s.reshape(NWIN, P).max(1)
        ck1 = c1s.reshape(NWIN, P).max(1)
        cm = np.zeros((TOTCH, P), np.float32)
        for w in range(NWIN):
            if TRIM:
                flat.reshape(TOTCH, P)[offs[w] + ck0[w]:offs[w] + K0s[w]] = -1
                flat.reshape(TOTCH, P)[offs[w] + K0s[w] + ck1[w]:
                                       offs[w + 1]] = -1
            cm[offs[w]:offs[w] + ck0[w]] = 1.0
            cm[offs[w] + K0s[w]:offs[w] + K0s[w] + ck1[w]] = 1.0
        flat[slot] = rowid.astype(np.int16)

        si16 = np.zeros((P, TOTCH * 8), np.int16)
        for w in range(NWIN):
            a, b = int(offs[w]), int(offs[w] + K0s[w])
            if b > a:
                si16[:, a * 8:b * 8] = _wrap16(flat[a * P:b * P])
            a, b = int(offs[w] + K0s[w]), int(offs[w + 1])
            if b > a:
                si16[:, a * 8:b * 8] = _wrap16(flat[a * P:b * P])

        # window-node half masks (m0=1.0 if node id < S0 -> self-loop slot in
        # half0; m1 = 1 - m0).  Pad labels (>= DPC) point at sentinel rows
        # whose a_dst is 0 -- either half works.
        node_of_label = np.zeros(NWIN * P, np.int64)
        node_of_label[:DPC] = order + c * DPC
        glob = node_of_label.reshape(NWIN, P).T       # [P, NWIN]
        m0h = (glob < S0).astype(np.float32)
        m0 = np.stack([m0h, 1.0 - m0h], axis=2)       # [P, NWIN, 2]

        in_maps.append({
            "xT": xT16,
            "wext": W16,
            "si16": si16,
            "m0": np.ascontiguousarray(m0.reshape(P, 2 * NWIN)),
            "cm": np.ascontiguousarray(cm.T.astype(BF16)),   # [P, TOTCH]
            "gbb": gbb,
            "sent": sent16,
        })
        out_perms.append(order + c * DPC)
    return in_maps, tuple(int(v) for v in K0s), tuple(int(v) for v in K1s), \
        TOTCH, out_perms, triv


# --------------------------------------------------------------------------
# Bass kernel builder (identical NEFF for all cores)
# --------------------------------------------------------------------------

def build_nc(cfg, K0s, K1s, TOTCH, triv=False, dbg=None):
    N, DIN, DH, H, C = cfg.N, cfg.DIN, cfg.DH, cfg.H, cfg.C
    ROW, ROW_T, NWIN, NT, NPAD, G1, KD = (cfg.ROW, cfg.ROW_T, cfg.NWIN,
                                          cfg.NT, cfg.NPAD, cfg.G1, cfg.KD)
    S0 = cfg.S0
    N1 = NPAD - S0
    KWs = [a + b for a, b in zip(K0s, K1s)]
    KWMAX = max(KWs)
    offs = [0]
    for v in KWs:
        offs.append(offs[-1] + v)
    NG1 = NT // G1
    BPG = (G1 + 2) // 3
    WOUT = cfg.WOUT

    nc = bacc.Bacc("TRN2", num_swdge_queues=4, dynamic_dma_scratch_size=32768)
    xT_d = nc.dram_tensor("xT", [DIN, NPAD], BF16_DT, kind="ExternalInput")
    w_d = nc.dram_tensor("wext", [DIN, ROW], BF16_DT, kind="ExternalInput")
    si_d = nc.dram_tensor("si16", [P, TOTCH * 8], I16, kind="ExternalInput")
    cm_d = nc.dram_tensor("cm", [P, TOTCH], BF16_DT, kind="ExternalInput")
    m0_d = nc.dram_tensor("m0", [P, 2 * NWIN], F32, kind="ExternalInput")
    gbb_d = nc.dram_tensor("gbb", [3, DH], F32, kind="ExternalInput")
    sent_d = nc.dram_tensor("sent", [2, ROW], BF16_DT, kind="ExternalInput")
    y_d = nc.dram_tensor("y", [NWIN * P, DH], F32, kind="ExternalOutput")
    htab0 = nc.dram_tensor("htab0", [S0 + P, ROW_T], BF16_DT, kind="Internal")
    htab1 = nc.dram_tensor("htab1", [N1 + P, ROW_T], BF16_DT, kind="Internal")

    qrr = [0]

    def next_q():
        q = qrr[0]
        qrr[0] = (q + 1) % 4
        return q

    MPBUFS = 3
    nc.gpsimd.load_library(library_config.mlp)
    with tile.TileContext(nc) as tc:
        with tc.tile_pool(name="const", bufs=1) as const, \
             tc.tile_pool(name="mp", bufs=MPBUFS) as mp, \
             tc.tile_pool(name="sip", bufs=MPBUFS) as sip:
            wt = const.tile([P, KD, ROW], BF16_DT)
            for k in range(KD):
                nc.sync.dma_start(out=wt[:, k, :], in_=w_d[k * P:(k + 1) * P, :])
            cm_t = const.tile([P, TOTCH], BF16_DT)
            nc.sync.dma_start(out=cm_t[:], in_=cm_d[:])
            m0_t = const.tile([P, 2 * NWIN], F32)
            nc.sync.dma_start(out=m0_t[:], in_=m0_d[:])
            gam_t = const.tile([P, DH], F32)
            bet_t = const.tile([P, DH], F32)
            bia_t = const.tile([P, DH], F32)
            for t, i in ((gam_t, 0), (bet_t, 1), (bia_t, 2)):
                a = gbb_d[i, :]
                src_ap = bass.AP(a.tensor, a.offset, [[0, P], [1, DH]])
                nc.gpsimd.dma_start(out=t[:], in_=src_ap)
            eps_t = const.tile([P, 1], F32)
            nc.vector.memset(eps_t[:], cfg.LN_EPS)
            neg_t = const.tile([P, 1], F32)
            nc.vector.memset(neg_t[:], cfg.NEG)
            sent_t = const.tile([2, ROW], BF16_DT)
            nc.sync.dma_start(out=sent_t[:], in_=sent_d[:])

            # ---- phase 1: htab[:, 0:136] = x @ W_ext ----
            nc.gpsimd.dma_start(out=htab0[S0:S0 + 1, 0:ROW], in_=sent_t[0:1, :])
            nc.gpsimd.dma_start(out=htab1[N1:N1 + 1, 0:ROW], in_=sent_t[1:2, :])
            with (
                tc.tile_pool(name="xp", bufs=2) as xp,
                tc.tile_pool(name="stg", bufs=2) as stg,
                tc.tile_pool(name="ps1", bufs=2, space="PSUM") as ps1,
            ):
                for g in range(NG1):
                    xk = xp.tile([P, KD, G1 * P], BF16_DT)
                    for k in range(KD):
                        nc.sync.dma_start(
                            out=xk[:, k, :],
                            in_=xT_d[k * P:(k + 1) * P,
                                     g * G1 * P:(g + 1) * G1 * P])
                    ps = ps1.tile([P, BPG, 512], F32, tag="ps1")
                    for i in range(G1):
                        pslice = ps[:, i // 3, (i % 3) * ROW:(i % 3 + 1) * ROW]
                        for k in range(KD):
                            nc.tensor.matmul(
                                pslice, lhsT=xk[:, k, i * P:(i + 1) * P],
                                rhs=wt[:, k, :],
                                start=(k == 0), stop=(k == KD - 1))
                    stage = stg.tile([P, G1, ROW], BF16_DT, tag="stage")
                    nc.scalar.copy(
                        out=stage[:].rearrange("p (b t) r -> p b t r", t=3),
                        in_=ps[:, :, 0:3 * ROW].rearrange(
                            "p b (t r) -> p b t r", r=ROW))
                    n0 = g * G1 * P
                    n1 = n0 + G1 * P
                    if n1 <= S0 or n0 >= S0:
                        t, o = (htab0, n0) if n1 <= S0 else (htab1, n0 - S0)
                        dst_ap = t[o:o + G1 * P, 0:ROW].rearrange(
                            "(b p) r -> p b r", p=P)
                        nc.gpsimd.dma_start(out=dst_ap, in_=stage[:])
                    else:
                        bs = (S0 - n0) // P   # boundary is 128-aligned
                        dst_ap = htab0[n0:S0, 0:ROW].rearrange(
                            "(b p) r -> p b r", p=P)
                        nc.gpsimd.dma_start(out=dst_ap, in_=stage[:, 0:bs, :])
                        dst_ap = htab1[0:n1 - S0, 0:ROW].rearrange(
                            "(b p) r -> p b r", p=P)
                        nc.gpsimd.dma_start(out=dst_ap, in_=stage[:, bs:, :])

            if dbg == "phase1":
                with tc.tile_pool(name="dbgp", bufs=2) as dbgp:
                    for w in range(NWIN):
                        t = dbgp.tile([P, DH], F32, tag="dbg")
                        nc.gpsimd.dma_start(out=t[:],
                                            in_=htab0[w * P:(w + 1) * P, 0:DH])
                        nc.gpsimd.dma_start(out=y_d[w * P:(w + 1) * P, :],
                                            in_=t[:])
            if dbg is None:
              # ---- phase 2 ----
              with (
                  tc.tile_pool(name="wp", bufs=2) as wp,
                  tc.tile_pool(name="fp", bufs=2) as fp,
                  tc.tile_pool(name="outp", bufs=2) as outp,
              ):
                  for _ in range(MPBUFS):
                      gz = mp.tile([P, KWMAX, ROW_T], BF16_DT, tag="gm")
                      nc.vector.memset(gz[:], 0.0)
                  ost = None
                  for w in range(NWIN):
                      K0, K1 = K0s[w], K1s[w]
                      KW = KWs[w]
                      off = offs[w]
                      si_t = sip.tile([P, KWMAX * 8], I16, tag="si")
                      nc.sync.dma_start(out=si_t[:, :KW * 8],
                                        in_=si_d[:, off * 8:(off + KW) * 8])
                      g_main = mp.tile([P, KWMAX, ROW_T], BF16_DT, tag="gm")
                      # gathers: one span per half, split at GSUB chunks
                      for base, kn, htb, span in ((0, K0, htab0, S0 + P),
                                                  (K0, K1, htab1, N1 + P)):
                          k = 0
                          while k < kn:
                              ke = min(k + cfg.GSUB, kn)
                              a = base + k
                              b = base + ke
                              nc.gpsimd.dma_gather(
                                  out_ap=g_main[:, a:b, :],
                                  in_ap=htb[0:span, :],
                                  idxs_ap=si_t[:, a * 8:b * 8],
                                  num_idxs=(b - a) * P,
                                  num_idxs_reg=(b - a) * P,
                                  elem_size=ROW_T,
                                  queue_num=next_q())
                              k = ke

                      # a_dst for the window's nodes: self-loop slot is chunk 0
                      # of the node's own half (blend by m0 mask)
                      adw = wp.tile([P, H], F32, tag="adw")
                      if K0 and K1:
                          nc.vector.tensor_scalar(
                              out=adw[:], in0=g_main[:, 0, DH + H:DH + 2 * H],
                              scalar1=m0_t[:, 2 * w:2 * w + 1], scalar2=None,
                              op0=mybir.AluOpType.mult)
                          nc.vector.scalar_tensor_tensor(
                              out=adw[:],
                              in0=g_main[:, K0, DH + H:DH + 2 * H],
                              scalar=m0_t[:, 2 * w + 1:2 * w + 2],
                              in1=adw[:], op0=mybir.AluOpType.mult,
                              op1=mybir.AluOpType.add)
                      else:
                          nc.vector.tensor_copy(
                              out=adw[:],
                              in_=g_main[:, 0, DH + H:DH + 2 * H])

                      # scores: sc = a_src + a_dst (bcast over chunks), then
                      # w = exp(leakyrelu(sc)) on the scalar engine
                      sc = wp.tile([P, KWMAX, H], F32, tag="sc")
                      a = adw[:]
                      ad_b = bass.AP(a.tensor, a.offset,
                                     [a.ap[0], [0, KW], a.ap[1]])
                      nc.vector.tensor_tensor(
                          out=sc[:, :KW, :], in0=g_main[:, :KW, DH:DH + H],
                          in1=ad_b, op=mybir.AluOpType.add)
                      sc2 = wp.tile([P, KWMAX, H], F32, tag="sc2")
                      nc.scalar.activation(out=sc2[:, :KW, :],
                                           in_=sc[:, :KW, :],
                                           func=mybir.ActivationFunctionType.Prelu,
                                           alpha=neg_t[:])
                      wf = wp.tile([P, KWMAX, H], BF16_DT, tag="wf")
                      nc.scalar.activation(out=wf[:, :KW, :], in_=sc2[:, :KW, :],
                                           func=mybir.ActivationFunctionType.Exp)
                      # zero w for this core's all-pad (trimmed) chunks
                      ca = cm_t[:, off:off + KW]
                      cm_b = bass.AP(ca.tensor, ca.offset,
                                     [ca.ap[0], ca.ap[1], [0, H]])
                      nc.vector.tensor_tensor(out=wf[:, :KW, :],
                                              in0=wf[:, :KW, :], in1=cm_b,
                                              op=mybir.AluOpType.mult)

                      # wrep = w broadcast to feature width, written into the
                      # pad half of the gathered rows (scalar engine, so the
                      # big DVE multiply below gets contiguous operands)
                      wrep = g_main[:, :KW, DH:2 * DH]
                      a2 = wf[:, :KW, :]
                      w_b = bass.AP(a2.tensor, a2.offset,
                                    [a2.ap[0], a2.ap[1], a2.ap[2], [0, C]])
                      nc.scalar.copy(
                          out=wrep.rearrange("p k (h c) -> p k h c", h=H),
                          in_=w_b)
                      # rhs = h * w, in place over the gathered h
                      rhs = g_main[:, :KW, 0:DH]
                      nc.vector.tensor_tensor(
                          out=rhs, in0=rhs, in1=wrep, op=mybir.AluOpType.mult)

                      # num = sum_k rhs: halving tree, contiguous operands
                      num = fp.tile([P, DH], F32, tag="num")
                      n = KW
                      while n > 2:
                          hh = n // 2          # fold tail onto head
                          ce = n - hh          # ceil
                          nc.vector.tensor_tensor(
                              out=g_main[:, :hh, 0:DH],
                              in0=g_main[:, :hh, 0:DH],
                              in1=g_main[:, ce:ce + hh, 0:DH],
                              op=mybir.AluOpType.add)
                          n = ce
                      if n == 2:
                          nc.vector.tensor_tensor(
                              out=num[:], in0=g_main[:, 0, 0:DH],
                              in1=g_main[:, 1, 0:DH],
                              op=mybir.AluOpType.add)
                      else:
                          nc.vector.tensor_copy(out=num[:],
                                                in_=g_main[:, 0, 0:DH])
                      # den = sum_k w (small strided reduce)
                      den = fp.tile([P, H], F32, tag="den")
                      nc.vector.tensor_reduce(
                          out=den[:],
                          in_=wf[:, :KW, :].rearrange("p k h -> p h k"),
                          axis=mybir.AxisListType.X, op=mybir.AluOpType.add)

                      nc.vector.tensor_scalar_add(out=den[:], in0=den[:],
                                                  scalar1=cfg.DEN_EPS)
                      nc.vector.reciprocal(out=den[:], in_=den[:])
                      y = fp.tile([P, DH], F32, tag="y")
                      da = den[:]
                      den_b = bass.AP(da.tensor, da.offset,
                                      [da.ap[0], da.ap[1], [0, C]])
                      nc.vector.tensor_tensor(
                          out=y[:].rearrange("p (h c) -> p h c", h=H),
                          in0=num[:].rearrange("p (h c) -> p h c", h=H),
                          in1=den_b, op=mybir.AluOpType.mult)
                      if not triv:
                          nc.vector.tensor_tensor(out=y[:], in0=y[:],
                                                  in1=bia_t[:],
                                                  op=mybir.AluOpType.add)
                      st = fp.tile([P, 6], F32, tag="st")
                      nc.vector.bn_stats(out=st[:], in_=y[:])
                      mv = fp.tile([P, 2], F32, tag="mv")
                      nc.vector.bn_aggr(out=mv[:], in_=st[:])
                      nc.scalar.activation(out=mv[:, 1:2], in_=mv[:, 1:2],
                                           func=mybir.ActivationFunctionType.Sqrt,
                                           bias=eps_t[:])
                      nc.vector.reciprocal(out=mv[:, 1:2], in_=mv[:, 1:2])
                      # z = (y - mu) * rstd on the scalar engine:
                      # scale = rstd, bias = -mu * rstd
                      nmr = fp.tile([P, 1], F32, tag="nmr")
                      nc.vector.scalar_tensor_tensor(
                          out=nmr[:], in0=mv[:, 0:1], scalar=-1.0,
                          in1=mv[:, 1:2], op0=mybir.AluOpType.mult,
                          op1=mybir.AluOpType.mult)
                      z = fp.tile([P, DH], F32, tag="z")
                      nc.scalar.activation(out=z[:], in_=y[:],
                                           func=mybir.ActivationFunctionType.Identity,
                                           scale=mv[:, 1:2], bias=nmr[:])
                      if not triv:
                          nc.vector.tensor_tensor(out=z[:], in0=z[:],
                                                  in1=gam_t[:],
                                                  op=mybir.AluOpType.mult)
                          nc.vector.tensor_tensor(out=z[:], in0=z[:],
                                                  in1=bet_t[:],
                                                  op=mybir.AluOpType.add)
                      # elu: out = (relu(z) - 1) + exp(min(z, 0))
                      zm = fp.tile([P, DH], F32, tag="zm")
                      nc.scalar.activation(out=zm[:], in_=z[:],
                                           func=mybir.ActivationFunctionType.Relu)
                      zn = fp.tile([P, DH], F32, tag="zn")
                      nc.vector.tensor_scalar(out=zn[:], in0=z[:], scalar1=0.0,
                                              scalar2=None,
                                              op0=mybir.AluOpType.min)
                      te = fp.tile([P, DH], F32, tag="te")
                      nc.scalar.activation(out=te[:], in_=zn[:],
                                           func=mybir.ActivationFunctionType.Exp)
                      if w % WOUT == 0:
                          ost = outp.tile([P, WOUT, DH], F32, tag="ost")
                      nc.vector.scalar_tensor_tensor(
                          out=ost[:, w % WOUT, :], in0=zm[:], scalar=-1.0,
                          in1=te[:], op0=mybir.AluOpType.add,
                          op1=mybir.AluOpType.add)
                      if w % WOUT == WOUT - 1 or w == NWIN - 1:
                          w0 = (w // WOUT) * WOUT
                          nb = w - w0 + 1
                          dst_ap = y_d[w0 * P:(w + 1) * P, :].rearrange(
                              "(b p) r -> p b r", p=P)
                          nc.scalar.dma_start(out=dst_ap, in_=ost[:, :nb, :])

    nc.compile()
    return nc


# --------------------------------------------------------------------------
# Entry point
# --------------------------------------------------------------------------

_CACHE = {}


def kernel(x, edge_index, W, att_src, att_dst, bias, ln_gamma, ln_beta,
           cfg=DEFAULT_CFG, trace=False, dbg=None):
    in_maps, K0s, K1s, TOTCH, out_perms, triv = host_prep(
        cfg, x, edge_index, W, att_src, att_dst, bias, ln_gamma, ln_beta)
    key = (cfg.N, cfg.E, K0s, K1s, TOTCH, triv, dbg)
    if key not in _CACHE:
        _CACHE[key] = build_nc(cfg, K0s, K1s, TOTCH, triv=triv, dbg=dbg)
    nc = _CACHE[key]
    r = run_bass_kernel_spmd(nc, in_maps, core_ids=list(range(cfg.NCORES)),
                             trace=trace)
    out = np.empty((cfg.N, cfg.DH), np.float32)
    for c in range(cfg.NCORES):
        out[out_perms[c]] = r.results[c]["y"][:cfg.D_PER_CORE]
    kernel.last_result = r
    return out


# revision 38
# speedup vs baseline: 1.2699x; 1.1240x over previous
"""GAT layer kernel for Trainium2, 8 NeuronCores.

Strategy (dst-sharded, zero collectives, identity-slot aggregation):
  - Host: append self-loops, split dst space into 8 equal ranges (one per
    core).  Per core, dst nodes are RELABELED in decreasing (in-degree+1)
    order; window w = labels [128w, 128w+128).  Edge (k-th incoming edge of
    label p, half hf) occupies gather slot (chunk, partition=p) where chunk
    enumerates k within the half-region.  Aggregation over a node's edges is
    then a plain sum over chunks at fixed partition -- NO onehot matmuls.
  - Phase 1 (replicated): htab[n] = [h(128) | a_src(4) | a_dst(4)] = x @
    W_ext via PE matmuls (bf16), rows strided 512B (dma_gather needs 256B
    multiples).  Tables split at S0=32512 so int16 gather indices reach every
    row; one sentinel row per table (h=0, a_src=-100) absorbs pad slots.
  - Phase 2 (per core, per window): one dma_gather per (window, half) on
    rotating SWDGE queues fetches rows into g[d-part, chunk, 512B].  Scores
    w = exp(leakyrelu(a_src + a_dst)) use the max-free softmax (bounded
    scores).  a_dst comes from the self-loop slot (chunk 0 of the node's own
    half).  num = sum_k w*h and den = sum_k w are DVE reduces; finalize is
    out = elu(layernorm(num/den + bias) * gamma + beta).  Output rows are in
    degree-sorted order; the host unpermutes.
"""

import numpy as np
import ml_dtypes

import concourse.bass as bass
import concourse.bacc as bacc
import concourse.mybir as mybir
import concourse.tile as tile
from concourse import library_config
from concourse.bass_utils import run_bass_kernel_spmd

BF16 = ml_dtypes.bfloat16
F32 = mybir.dt.float32
BF16_DT = mybir.dt.bfloat16
I16 = mybir.dt.int16

P = 128


class Cfg:
    def __init__(self, N=50000, E=1600000, DIN=256, DH=128, H=4, NCORES=8):
        self.N, self.E, self.DIN, self.DH, self.H = N, E, DIN, DH, H
        self.C = DH // H
        self.NCORES = NCORES
        self.ROW = DH + 2 * H               # 136 payload cols
        self.ROW_T = 256                    # stored row (512B stride)
        self.D_PER_CORE = N // NCORES       # 6250
        self.NWIN = (self.D_PER_CORE + P - 1) // P   # 49
        self.G1 = 12                        # node tiles per phase-1 group
        nt = (N + P - 1) // P
        self.NT = ((nt + self.G1 - 1) // self.G1) * self.G1   # 396
        self.NPAD = self.NT * P             # 50688
        self.KD = (DIN + P - 1) // P        # matmul k-chunks (2)
        self.WOUT = 4                       # windows batched per output store
        self.S0 = 32512                     # htab0 rows (254*128, idx<=32512)
        self.GSUB = 8                       # max chunks per gather op

    NEG = 0.2
    LN_EPS = 1e-5
    DEN_EPS = 1e-20
    SENT_A = -100.0


DEFAULT_CFG = Cfg()
# Negative-idx suffix trimming wedges the device (ring accounting vs the
# static num_idxs_reg) -- keep disabled.
TRIM = False


def _wrap16(flat):
    """int16 index list -> dma_gather idxs layout [128, n/16]."""
    n = len(flat)
    a = flat.reshape(n // 16, 16).T          # [16, n/16]
    return np.tile(a, (8, 1))                # replicated to 128 partitions


# --------------------------------------------------------------------------
# Host-side preparation (layout only; all FLOPs on x stay on device)
# --------------------------------------------------------------------------

def host_prep(cfg, x, edge_index, W, att_src, att_dst, bias, ln_gamma, ln_beta):
    N, DIN, DH, H, C = cfg.N, cfg.DIN, cfg.DH, cfg.H, cfg.C
    NC, DPC, NWIN, S0 = cfg.NCORES, cfg.D_PER_CORE, cfg.NWIN, cfg.S0
    N1 = cfg.NPAD - S0                       # htab1 node rows

    x = np.asarray(x, np.float32)
    W = np.asarray(W, np.float32)
    att_src = np.asarray(att_src, np.float32)
    att_dst = np.asarray(att_dst, np.float32)

    Msrc = np.zeros((DH, H), np.float32)
    Mdst = np.zeros((DH, H), np.float32)
    for h in range(H):
        Msrc[h * C:(h + 1) * C, h] = att_src[h]
        Mdst[h * C:(h + 1) * C, h] = att_dst[h]
    W_ext = np.concatenate([W, W @ Msrc, W @ Mdst], axis=1)  # [DIN, ROW]
    W16 = np.ascontiguousarray(W_ext).astype(BF16)

    xT = np.zeros((DIN, cfg.NPAD), np.float32)
    xT[:, :N] = x.T
    xT16 = xT.astype(BF16)

    src = np.asarray(edge_index[0], np.int64)
    dst = np.asarray(edge_index[1], np.int64)

    gbb = np.stack([np.asarray(ln_gamma, np.float32),
                    np.asarray(ln_beta, np.float32),
                    np.asarray(bias, np.float32)], 0)
    # trivial affine params (the common init) let the kernel skip 3 DVE ops
    triv = bool(np.all(gbb[0] == 1.0) and np.all(gbb[1] == 0.0)
                and np.all(gbb[2] == 0.0))
    # sentinel rows: h = 0, a_src = SENT_A, a_dst = 0
    sent = np.zeros((2, cfg.ROW), np.float32)
    sent[:, DH:DH + H] = cfg.SENT_A
    sent16 = sent.astype(BF16)

    # ---- per-core edge layout ----
    core_of = dst // DPC
    per_core = []   # (order, c0, c1) per core
    for c in range(NC):
        m = core_of == c
        s_c = src[m]
        d_loc = dst[m] - c * DPC
        # self-loops for this core's dst range
        own = np.arange(c * DPC, (c + 1) * DPC, dtype=np.int64)
        s_all = np.concatenate([own, s_c])
        d_all = np.concatenate([own - c * DPC, d_loc])
        is_self = np.zeros(len(s_all), np.int8)
        is_self[:DPC] = 1      # used to force self-loop to slot k=0
        hf = (s_all >= S0).astype(np.int64)
        c0 = np.bincount(d_all[hf == 0], minlength=DPC)
        c1 = np.bincount(d_all[hf == 1], minlength=DPC)
        per_core.append([None, c0, c1, s_all, d_all, hf, is_self])

    # window packing: band by c0 (width B), sort by c1 within band -- windows
    # get a tight max-c0 and max-c1.  Scan B for the best global chunk count.
    def pack(B):
        K0s = np.zeros(NWIN, np.int64)
        K1s = np.zeros(NWIN, np.int64)
        orders = []
        for c in range(NC):
            c0, c1 = per_core[c][1], per_core[c][2]
            order = np.lexsort((-c1, -(c0 // B)))
            orders.append(order)
            c0s = np.zeros(NWIN * P, np.int64)
            c1s = np.zeros(NWIN * P, np.int64)
            c0s[:DPC] = c0[order]
            c1s[:DPC] = c1[order]
            K0s = np.maximum(K0s, c0s.reshape(NWIN, P).max(1))
            K1s = np.maximum(K1s, c1s.reshape(NWIN, P).max(1))
        return (K0s + K1s).sum(), orders, K0s, K1s

    best = min((pack(B) for B in (3, 4, 5, 6, 8, 10)), key=lambda t: t[0])
    _, orders, K0s, K1s = best
    for c in range(NC):
        per_core[c][0] = orders[c]
    KWs = K0s + K1s
    offs = np.zeros(NWIN + 1, np.int64)
    np.cumsum(KWs, out=offs[1:])
    TOTCH = int(offs[-1])

    in_maps = []
    out_perms = []
    for c in range(NC):
        order, c0, c1, s_all, d_all, hf, is_self = per_core[c]
        label_of = np.empty(DPC, np.int64)
        label_of[order] = np.arange(DPC)
        lab = label_of[d_all]                      # label per edge
        w_of = lab >> 7
        p_of = lab & 127
        # position of each edge within its (label, half) group, self first
        key = ((lab * 2 + hf) << 1) | (1 - is_self).astype(np.int64)
        eo = np.argsort(key, kind="stable")
        ks = key[eo] >> 1                          # group id = lab*2+hf
        starts = np.searchsorted(ks, np.arange(DPC * 2) * 1)
        # position within group
        grp_start = starts[ks]
        pos = np.arange(len(eo)) - grp_start
        # chunk index within window
        k0w = K0s[w_of[eo]]
        chunk = np.where(hf[eo] == 0, pos, k0w + pos)
        gchunk = offs[w_of[eo]] + chunk
        slot = gchunk * P + p_of[eo]
        rowid = np.where(hf[eo] == 0, s_all[eo], s_all[eo] - S0)

        flat = np.empty(TOTCH * P, np.int16)
        # defaults: sentinel of the chunk's half; chunks beyond this core's
        # own per-window max count are all-pad -> negative idx (the SWDGE
        # ucode suffix-trims them, skipping both descriptors and fetch)
        half1_chunk = np.zeros(TOTCH, bool)
        for w in range(NWIN):
            half1_chunk[offs[w] + K0s[w]:offs[w + 1]] = True
        flat.reshape(TOTCH, P)[~half1_chunk] = S0        # sentinel htab0
        flat.reshape(TOTCH, P)[half1_chunk] = N1         # sentinel htab1
        c0s = np.zeros(NWIN * P, np.int64)
        c1s = np.zeros(NWIN * P, np.int64)
        c0s[:DPC] = c0[order]
        c1s[:DPC] = c1[order]
        ck0 = c0s.reshape(NWIN, P).max(1)
        ck1 = c1s.reshape(NWIN, P).max(1)
        cm = np.zeros((TOTCH, P), np.float32)
        for w in range(NWIN):
            if TRIM:
                flat.reshape(TOTCH, P)[offs[w] + ck0[w]:offs[w] + K0s[w]] = -1
                flat.reshape(TOTCH, P)[offs[w] + K0s[w] + ck1[w]:
                                       offs[w + 1]] = -1
            cm[offs[w]:offs[w] + ck0[w]] = 1.0
            cm[offs[w] + K0s[w]:offs[w] + K0s[w] + ck1[w]] = 1.0
        flat[slot] = rowid.astype(np.int16)

        si16 = np.zeros((P, TOTCH * 8), np.int16)
        for w in range(NWIN):
            a, b = int(offs[w]), int(offs[w] + K0s[w])
            if b > a:
                si16[:, a * 8:b * 8] = _wrap16(flat[a * P:b * P])
            a, b = int(offs[w] + K0s[w]), int(offs[w + 1])
            if b > a:
                si16[:, a * 8:b * 8] = _wrap16(flat[a * P:b * P])

        # window-node half masks (m0=1.0 if node id < S0 -> self-loop slot in
        # half0; m1 = 1 - m0), replicated across heads so the blend runs as
        # plain tensor_tensor (pointer-scalar DVE ops are ~40x slower).
        # Pad labels (>= DPC) point at sentinel rows (a_dst 0, either half).
        node_of_label = np.zeros(NWIN * P, np.int64)
        node_of_label[:DPC] = order + c * DPC
        glob = node_of_label.reshape(NWIN, P).T       # [P, NWIN]
        m0h = (glob < S0).astype(np.float32)
        m0 = np.stack([m0h, 1.0 - m0h], axis=2)       # [P, NWIN, 2]
        m0 = np.repeat(m0[:, :, :, None], H, axis=3)  # [P, NWIN, 2, H]

        in_maps.append({
            "xT": xT16,
            "wext": W16,
            "si16": si16,
            "m0": np.ascontiguousarray(m0.reshape(P, 2 * NWIN * H)),
            "cm": np.ascontiguousarray(cm.T.astype(BF16)),   # [P, TOTCH]
            "gbb": gbb,
            "sent": sent16,
        })
        out_perms.append(order + c * DPC)
    return in_maps, tuple(int(v) for v in K0s), tuple(int(v) for v in K1s), \
        TOTCH, out_perms, triv


# --------------------------------------------------------------------------
# Bass kernel builder (identical NEFF for all cores)
# --------------------------------------------------------------------------

def build_nc(cfg, K0s, K1s, TOTCH, triv=False, dbg=None):
    N, DIN, DH, H, C = cfg.N, cfg.DIN, cfg.DH, cfg.H, cfg.C
    ROW, ROW_T, NWIN, NT, NPAD, G1, KD = (cfg.ROW, cfg.ROW_T, cfg.NWIN,
                                          cfg.NT, cfg.NPAD, cfg.G1, cfg.KD)
    S0 = cfg.S0
    N1 = NPAD - S0
    KWs = [a + b for a, b in zip(K0s, K1s)]
    KWMAX = max(KWs)
    offs = [0]
    for v in KWs:
        offs.append(offs[-1] + v)
    NG1 = NT // G1
    BPG = (G1 + 2) // 3
    WOUT = cfg.WOUT

    nc = bacc.Bacc("TRN2", num_swdge_queues=4, dynamic_dma_scratch_size=32768)
    xT_d = nc.dram_tensor("xT", [DIN, NPAD], BF16_DT, kind="ExternalInput")
    w_d = nc.dram_tensor("wext", [DIN, ROW], BF16_DT, kind="ExternalInput")
    si_d = nc.dram_tensor("si16", [P, TOTCH * 8], I16, kind="ExternalInput")
    cm_d = nc.dram_tensor("cm", [P, TOTCH], BF16_DT, kind="ExternalInput")
    m0_d = nc.dram_tensor("m0", [P, 2 * NWIN * H], F32, kind="ExternalInput")
    gbb_d = nc.dram_tensor("gbb", [3, DH], F32, kind="ExternalInput")
    sent_d = nc.dram_tensor("sent", [2, ROW], BF16_DT, kind="ExternalInput")
    y_d = nc.dram_tensor("y", [NWIN * P, DH], F32, kind="ExternalOutput")
    htab0 = nc.dram_tensor("htab0", [S0 + P, ROW_T], BF16_DT, kind="Internal")
    htab1 = nc.dram_tensor("htab1", [N1 + P, ROW_T], BF16_DT, kind="Internal")

    qrr = [0]

    def next_q():
        q = qrr[0]
        qrr[0] = (q + 1) % 4
        return q

    MPBUFS = 3
    nc.gpsimd.load_library(library_config.mlp)
    with tile.TileContext(nc) as tc:
        with tc.tile_pool(name="const", bufs=1) as const, \
             tc.tile_pool(name="mp", bufs=MPBUFS) as mp, \
             tc.tile_pool(name="sip", bufs=MPBUFS) as sip:
            wt = const.tile([P, KD, ROW], BF16_DT)
            for k in range(KD):
                nc.sync.dma_start(out=wt[:, k, :], in_=w_d[k * P:(k + 1) * P, :])
            cm_t = const.tile([P, TOTCH], BF16_DT)
            nc.sync.dma_start(out=cm_t[:], in_=cm_d[:])
            m0_t = const.tile([P, NWIN, 2, H], F32)
            nc.sync.dma_start(out=m0_t[:], in_=m0_d[:].rearrange(
                "p (w t h) -> p w t h", t=2, h=H))
            gam_t = const.tile([P, DH], F32)
            bet_t = const.tile([P, DH], F32)
            bia_t = const.tile([P, DH], F32)
            for t, i in ((gam_t, 0), (bet_t, 1), (bia_t, 2)):
                a = gbb_d[i, :]
                src_ap = bass.AP(a.tensor, a.offset, [[0, P], [1, DH]])
                nc.gpsimd.dma_start(out=t[:], in_=src_ap)
            eps_t = const.tile([P, 1], F32)
            nc.vector.memset(eps_t[:], cfg.LN_EPS)
            sent_t = const.tile([2, ROW], BF16_DT)
            nc.sync.dma_start(out=sent_t[:], in_=sent_d[:])

            # ---- phase 1: htab[:, 0:136] = x @ W_ext ----
            nc.gpsimd.dma_start(out=htab0[S0:S0 + 1, 0:ROW], in_=sent_t[0:1, :])
            nc.gpsimd.dma_start(out=htab1[N1:N1 + 1, 0:ROW], in_=sent_t[1:2, :])
            with (
                tc.tile_pool(name="xp", bufs=2) as xp,
                tc.tile_pool(name="stg", bufs=2) as stg,
                tc.tile_pool(name="ps1", bufs=2, space="PSUM") as ps1,
            ):
                for g in range(NG1):
                    xk = xp.tile([P, KD, G1 * P], BF16_DT)
                    for k in range(KD):
                        nc.sync.dma_start(
                            out=xk[:, k, :],
                            in_=xT_d[k * P:(k + 1) * P,
                                     g * G1 * P:(g + 1) * G1 * P])
                    ps = ps1.tile([P, BPG, 512], F32, tag="ps1")
                    for i in range(G1):
                        pslice = ps[:, i // 3, (i % 3) * ROW:(i % 3 + 1) * ROW]
                        for k in range(KD):
                            nc.tensor.matmul(
                                pslice, lhsT=xk[:, k, i * P:(i + 1) * P],
                                rhs=wt[:, k, :],
                                start=(k == 0), stop=(k == KD - 1))
                    stage = stg.tile([P, G1, ROW], BF16_DT, tag="stage")
                    nc.scalar.copy(
                        out=stage[:].rearrange("p (b t) r -> p b t r", t=3),
                        in_=ps[:, :, 0:3 * ROW].rearrange(
                            "p b (t r) -> p b t r", r=ROW))
                    n0 = g * G1 * P
                    n1 = n0 + G1 * P
                    if n1 <= S0 or n0 >= S0:
                        t, o = (htab0, n0) if n1 <= S0 else (htab1, n0 - S0)
                        dst_ap = t[o:o + G1 * P, 0:ROW].rearrange(
                            "(b p) r -> p b r", p=P)
                        nc.gpsimd.dma_start(out=dst_ap, in_=stage[:])
                    else:
                        bs = (S0 - n0) // P   # boundary is 128-aligned
                        dst_ap = htab0[n0:S0, 0:ROW].rearrange(
                            "(b p) r -> p b r", p=P)
                        nc.gpsimd.dma_start(out=dst_ap, in_=stage[:, 0:bs, :])
                        dst_ap = htab1[0:n1 - S0, 0:ROW].rearrange(
                            "(b p) r -> p b r", p=P)
                        nc.gpsimd.dma_start(out=dst_ap, in_=stage[:, bs:, :])

            if dbg == "phase1":
                with tc.tile_pool(name="dbgp", bufs=2) as dbgp:
                    for w in range(NWIN):
                        t = dbgp.tile([P, DH], F32, tag="dbg")
                        nc.gpsimd.dma_start(out=t[:],
                                            in_=htab0[w * P:(w + 1) * P, 0:DH])
                        nc.gpsimd.dma_start(out=y_d[w * P:(w + 1) * P, :],
                                            in_=t[:])
            if dbg is None:
              # ---- phase 2 ----
              with (
                  tc.tile_pool(name="wp", bufs=2) as wp,
                  tc.tile_pool(name="fp", bufs=2) as fp,
                  tc.tile_pool(name="outp", bufs=2) as outp,
              ):
                  for _ in range(MPBUFS):
                      gz = mp.tile([P, KWMAX, ROW_T], BF16_DT, tag="gm")
                      nc.vector.memset(gz[:], 0.0)
                  ost = None
                  for w in range(NWIN):
                      K0, K1 = K0s[w], K1s[w]
                      KW = KWs[w]
                      off = offs[w]
                      si_t = sip.tile([P, KWMAX * 8], I16, tag="si")
                      nc.sync.dma_start(out=si_t[:, :KW * 8],
                                        in_=si_d[:, off * 8:(off + KW) * 8])
                      g_main = mp.tile([P, KWMAX, ROW_T], BF16_DT, tag="gm")
                      # gathers: one span per half, split at GSUB chunks
                      for base, kn, htb, span in ((0, K0, htab0, S0 + P),
                                                  (K0, K1, htab1, N1 + P)):
                          k = 0
                          while k < kn:
                              ke = min(k + cfg.GSUB, kn)
                              a = base + k
                              b = base + ke
                              nc.gpsimd.dma_gather(
                                  out_ap=g_main[:, a:b, :],
                                  in_ap=htb[0:span, :],
                                  idxs_ap=si_t[:, a * 8:b * 8],
                                  num_idxs=(b - a) * P,
                                  num_idxs_reg=(b - a) * P,
                                  elem_size=ROW_T,
                                  queue_num=next_q())
                              k = ke

                      # a_dst for the window's nodes: self-loop slot is chunk 0
                      # of the node's own half (blend by m0 mask)
                      adw = wp.tile([P, H], F32, tag="adw")
                      if K0 and K1:
                          nc.vector.tensor_tensor(
                              out=adw[:], in0=g_main[:, 0, DH + H:DH + 2 * H],
                              in1=m0_t[:, w, 0, :], op=mybir.AluOpType.mult)
                          ad1 = wp.tile([P, H], F32, tag="ad1")
                          nc.vector.tensor_tensor(
                              out=ad1[:],
                              in0=g_main[:, K0, DH + H:DH + 2 * H],
                              in1=m0_t[:, w, 1, :], op=mybir.AluOpType.mult)
                          nc.vector.tensor_tensor(
                              out=adw[:], in0=adw[:], in1=ad1[:],
                              op=mybir.AluOpType.add)
                      else:
                          nc.vector.tensor_copy(
                              out=adw[:],
                              in_=g_main[:, 0, DH + H:DH + 2 * H])

                      # scores: sc = a_src + a_dst (bcast over chunks);
                      # w = exp(leakyrelu(sc)), exp on the scalar engine
                      sc = wp.tile([P, KWMAX, H], F32, tag="sc")
                      a = adw[:]
                      ad_b = bass.AP(a.tensor, a.offset,
                                     [a.ap[0], [0, KW], a.ap[1]])
                      nc.vector.tensor_tensor(
                          out=sc[:, :KW, :], in0=g_main[:, :KW, DH:DH + H],
                          in1=ad_b, op=mybir.AluOpType.add)
                      sc2 = wp.tile([P, KWMAX, H], F32, tag="sc2")
                      nc.vector.tensor_scalar_mul(out=sc2[:, :KW, :],
                                                  in0=sc[:, :KW, :],
                                                  scalar1=cfg.NEG)
                      nc.vector.tensor_tensor(out=sc2[:, :KW, :],
                                              in0=sc[:, :KW, :],
                                              in1=sc2[:, :KW, :],
                                              op=mybir.AluOpType.max)
                      wf = wp.tile([P, KWMAX, H], BF16_DT, tag="wf")
                      nc.scalar.activation(out=wf[:, :KW, :], in_=sc2[:, :KW, :],
                                           func=mybir.ActivationFunctionType.Exp)
                      # zero w for this core's all-pad (trimmed) chunks
                      ca = cm_t[:, off:off + KW]
                      cm_b = bass.AP(ca.tensor, ca.offset,
                                     [ca.ap[0], ca.ap[1], [0, H]])
                      nc.vector.tensor_tensor(out=wf[:, :KW, :],
                                              in0=wf[:, :KW, :], in1=cm_b,
                                              op=mybir.AluOpType.mult)

                      # wrep = w broadcast to feature width, written into the
                      # pad half of the gathered rows (scalar engine, so the
                      # big DVE multiply below gets contiguous operands)
                      wrep = g_main[:, :KW, DH:2 * DH]
                      a2 = wf[:, :KW, :]
                      w_b = bass.AP(a2.tensor, a2.offset,
                                    [a2.ap[0], a2.ap[1], a2.ap[2], [0, C]])
                      nc.scalar.copy(
                          out=wrep.rearrange("p k (h c) -> p k h c", h=H),
                          in_=w_b)
                      # rhs = h * w, in place over the gathered h
                      rhs = g_main[:, :KW, 0:DH]
                      nc.vector.tensor_tensor(
                          out=rhs, in0=rhs, in1=wrep, op=mybir.AluOpType.mult)

                      # num = sum_k rhs: halving tree, contiguous operands
                      num = fp.tile([P, DH], F32, tag="num")
                      n = KW
                      while n > 2:
                          hh = n // 2          # fold tail onto head
                          ce = n - hh          # ceil
                          nc.vector.tensor_tensor(
                              out=g_main[:, :hh, 0:DH],
                              in0=g_main[:, :hh, 0:DH],
                              in1=g_main[:, ce:ce + hh, 0:DH],
                              op=mybir.AluOpType.add)
                          n = ce
                      if n == 2:
                          nc.vector.tensor_tensor(
                              out=num[:], in0=g_main[:, 0, 0:DH],
                              in1=g_main[:, 1, 0:DH],
                              op=mybir.AluOpType.add)
                      else:
                          nc.vector.tensor_copy(out=num[:],
                                                in_=g_main[:, 0, 0:DH])
                      # den = sum_k w (small strided reduce)
                      den = fp.tile([P, H], F32, tag="den")
                      nc.vector.tensor_reduce(
                          out=den[:],
                          in_=wf[:, :KW, :].rearrange("p k h -> p h k"),
                          axis=mybir.AxisListType.X, op=mybir.AluOpType.add)

                      nc.vector.tensor_scalar_add(out=den[:], in0=den[:],
                                                  scalar1=cfg.DEN_EPS)
                      nc.vector.reciprocal(out=den[:], in_=den[:])
                      j = w % WOUT
                      if j == 0:
                          yb = outp.tile([P, WOUT, DH], F32, tag="yb")
                          mvb = outp.tile([P, WOUT, 2], F32, tag="mvb")
                      da = den[:]
                      den_b = bass.AP(da.tensor, da.offset,
                                      [da.ap[0], da.ap[1], [0, C]])
                      nc.vector.tensor_tensor(
                          out=yb[:, j, :].rearrange("p (h c) -> p h c", h=H),
                          in0=num[:].rearrange("p (h c) -> p h c", h=H),
                          in1=den_b, op=mybir.AluOpType.mult)
                      if not triv:
                          nc.vector.tensor_tensor(out=yb[:, j, :],
                                                  in0=yb[:, j, :],
                                                  in1=bia_t[:],
                                                  op=mybir.AluOpType.add)
                      st = fp.tile([P, 6], F32, tag="st")
                      nc.vector.bn_stats(out=st[:], in_=yb[:, j, :])
                      nc.vector.bn_aggr(out=mvb[:, j, :], in_=st[:])
                      if j == WOUT - 1 or w == NWIN - 1:
                          # batched LN + elu for the WOUT windows (amortizes
                          # the scalar-engine activation-table reloads)
                          nb = j + 1
                          nc.scalar.activation(
                              out=mvb[:, :nb, 1:2], in_=mvb[:, :nb, 1:2],
                              func=mybir.ActivationFunctionType.Sqrt,
                              bias=eps_t[:])
                          nc.vector.reciprocal(out=mvb[:, :nb, 1:2],
                                               in_=mvb[:, :nb, 1:2])
                          nmr = fp.tile([P, WOUT], F32, tag="nmr")
                          nc.vector.scalar_tensor_tensor(
                              out=nmr[:, :nb], in0=mvb[:, :nb, 0],
                              scalar=-1.0, in1=mvb[:, :nb, 1],
                              op0=mybir.AluOpType.mult,
                              op1=mybir.AluOpType.mult)
                          # z = y * rstd + (-mu * rstd)
                          zb = fp.tile([P, WOUT, DH], F32, tag="zb")
                          ra = mvb[:, :nb, 1:2]
                          rstd_b = bass.AP(ra.tensor, ra.offset,
                                           [ra.ap[0], ra.ap[1], [0, DH]])
                          nc.vector.tensor_tensor(
                              out=zb[:, :nb, :], in0=yb[:, :nb, :],
                              in1=rstd_b, op=mybir.AluOpType.mult)
                          na = nmr[:, :nb]
                          nmr_b = bass.AP(na.tensor, na.offset,
                                          [na.ap[0], na.ap[1], [0, DH]])
                          nc.vector.tensor_tensor(
                              out=zb[:, :nb, :], in0=zb[:, :nb, :],
                              in1=nmr_b, op=mybir.AluOpType.add)
                          if not triv:
                              ga = gam_t[:]
                              gam_b = bass.AP(ga.tensor, ga.offset,
                                              [ga.ap[0], [0, nb], ga.ap[1]])
                              ba = bet_t[:]
                              bet_b = bass.AP(ba.tensor, ba.offset,
                                              [ba.ap[0], [0, nb], ba.ap[1]])
                              nc.vector.tensor_tensor(
                                  out=zb[:, :nb, :], in0=zb[:, :nb, :],
                                  in1=gam_b, op=mybir.AluOpType.mult)
                              nc.vector.tensor_tensor(
                                  out=zb[:, :nb, :], in0=zb[:, :nb, :],
                                  in1=bet_b, op=mybir.AluOpType.add)
                          # elu: out = (relu(z) - 1) + exp(min(z, 0))
                          zmb = fp.tile([P, WOUT, DH], F32, tag="zmb")
                          nc.scalar.activation(
                              out=zmb[:, :nb, :], in_=zb[:, :nb, :],
                              func=mybir.ActivationFunctionType.Relu)
                          nc.vector.tensor_scalar(out=zb[:, :nb, :],
                                                  in0=zb[:, :nb, :],
                                                  scalar1=0.0, scalar2=None,
                                                  op0=mybir.AluOpType.min)
                          teb = fp.tile([P, WOUT, DH], F32, tag="teb")
                          nc.scalar.activation(
                              out=teb[:, :nb, :], in_=zb[:, :nb, :],
                              func=mybir.ActivationFunctionType.Exp)
                          ost = outp.tile([P, WOUT, DH], F32, tag="ost")
                          nc.vector.scalar_tensor_tensor(
                              out=ost[:, :nb, :], in0=zmb[:, :nb, :],
                              scalar=-1.0, in1=teb[:, :nb, :],
                              op0=mybir.AluOpType.add,
                              op1=mybir.AluOpType.add)
                          w0 = (w // WOUT) * WOUT
                          dst_ap = y_d[w0 * P:(w + 1) * P, :].rearrange(
                              "(b p) r -> p b r", p=P)
                          nc.scalar.dma_start(out=dst_ap, in_=ost[:, :nb, :])

    nc.compile()
    return nc


# --------------------------------------------------------------------------
# Entry point
# --------------------------------------------------------------------------

_CACHE = {}


def kernel(x, edge_index, W, att_src, att_dst, bias, ln_gamma, ln_beta,
           cfg=DEFAULT_CFG, trace=False, dbg=None):
    in_maps, K0s, K1s, TOTCH, out_perms, triv = host_prep(
        cfg, x, edge_index, W, att_src, att_dst, bias, ln_gamma, ln_beta)
    key = (cfg.N, cfg.E, K0s, K1s, TOTCH, triv, dbg)
    if key not in _CACHE:
        _CACHE[key] = build_nc(cfg, K0s, K1s, TOTCH, triv=triv, dbg=dbg)
    nc = _CACHE[key]
    r = run_bass_kernel_spmd(nc, in_maps, core_ids=list(range(cfg.NCORES)),
                             trace=trace)
    out = np.empty((cfg.N, cfg.DH), np.float32)
    for c in range(cfg.NCORES):
        out[out_perms[c]] = r.results[c]["y"][:cfg.D_PER_CORE]
    kernel.last_result = r
    return out
